# revision 1
# baseline (speedup 1.0000x reference)
"""Trainium2 Bass kernel for nn_Luong_61684320305412 (bidirectional masked
softmax attention, B=8, L0=L1=2048, D=256).

Sharding: data-parallel over batch B across the 8 NeuronCores (one batch
element per core). Per core:

    S      = q0 @ q1^T * (1/256) + NEG * mask0[:,None]*mask1[None,:]
    E      = exp(S)            (no max-subtraction needed: |S_unmasked| << 80,
                                masked entries underflow to exactly 0)
    out0   = (E @ q1) * (1/16) / rowsum(E)[:, None]
    out1   = (E^T @ q0) * (1/16) / colsum(E)[None, :]^T

Implementation notes:
  - The mask outer product is folded into the score matmul as a rank-1
    augmented contraction: an extra K=1 matmul with lhsT = -2^17*mask_l,
    rhs = +2^17*mask_r, so exp sees -2^26 on masked entries -> exactly 0.
  - Row/col sums come from an appended ones-column in the rhs of the
    out-matmuls (psum column D holds the softmax denominator).
  - E is needed with both orientations on the partition axis; we compute
    S twice (S and S^T) from transposed copies of q0/q1 rather than
    transposing the 2048x2048 E.
  - All matmuls use float32r (full-rate fp32 path, 1 cycle/row for N>=256).
  - L1 (resp. L0) is processed in halves so only half of E (8 MB) is
    resident in SBUF at a time.
"""

import math
from contextlib import ExitStack

import numpy as np

import concourse.bass as bass
import concourse.tile as tile
from concourse import bacc, mybir
from concourse.bass_utils import run_bass_kernel_spmd
from concourse.masks import make_identity

P = 128
B = 8
L = 2048          # L0 == L1
D = 256
T = L // P        # 16 row tiles
DC = D // P       # 2 contraction chunks of 128
HALF = L // 2     # 1024
NCHUNK = 512      # psum bank width in fp32
AUGW = D + 2      # 258: q-tiles augmented with two ones columns (even N for fp32r)
MASKC = 131072.0  # 2^17; (-2^17 m0)*(2^17 m1)/256 = -2^26 -> exp underflows to 0
SCALE2 = 1.0 / 256.0   # applied to scores inside exp
SCALE1 = 1.0 / 16.0    # applied to the averaged values at the end

f32 = mybir.dt.float32
f32r = mybir.dt.float32r
i32 = mybir.dt.int32
MUL = mybir.AluOpType.mult
EXP = mybir.ActivationFunctionType.Exp


def _emit(tc: tile.TileContext, ctx: ExitStack, io: dict):
    nc = tc.nc
    q0, q1, m0, m1 = io["q0"], io["q1"], io["mask0"], io["mask1"]
    out0, out1 = io["out0"], io["out1"]

    consts = ctx.enter_context(tc.tile_pool(name="consts", bufs=1))
    qaug = ctx.enter_context(tc.tile_pool(name="qaug", bufs=1))
    qT = ctx.enter_context(tc.tile_pool(name="qT", bufs=1))
    e_pool = ctx.enter_context(tc.tile_pool(name="e", bufs=18))
    outp = ctx.enter_context(tc.tile_pool(name="outp", bufs=4))
    small = ctx.enter_context(tc.tile_pool(name="small", bufs=4))
    t_psum = ctx.enter_context(tc.tile_pool(name="t_psum", bufs=2, space="PSUM"))
    s_psum = ctx.enter_context(tc.tile_pool(name="s_psum", bufs=2, space="PSUM"))
    o_psum = ctx.enter_context(tc.tile_pool(name="o_psum", bufs=2, space="PSUM"))

    # ---- load q0/q1 into augmented layout [p, t, D+2] (ones columns at D, D+1;
    # width D+2=258 keeps the fp32r matmul moving-dim even) ----
    q0a = qaug.tile([P, T, AUGW], f32r)
    q1a = qaug.tile([P, T, AUGW], f32r)
    nc.sync.dma_start(
        out=q0a[:, :, 0:D], in_=q0.rearrange("(t p) d -> p t d", p=P).bitcast(f32r)
    )
    nc.sync.dma_start(
        out=q1a[:, :, 0:D], in_=q1.rearrange("(t p) d -> p t d", p=P).bitcast(f32r)
    )
    # memset can't write f32r; stage ones in f32 and round via tensor_copy
    ones_f = consts.tile([P, T, 2], f32)
    nc.vector.memset(ones_f, 1.0)
    nc.vector.tensor_copy(out=q0a[:, :, D:AUGW], in_=ones_f)
    nc.vector.tensor_copy(out=q1a[:, :, D:AUGW], in_=ones_f)

    # ---- masks: int32 [L] -> f32 rows scaled by -+2^17 ----
    # (separate [1, L] tiles: matmul operands must start at partition 0)
    m0i = consts.tile([1, L], i32)
    m1i = consts.tile([1, L], i32)
    nc.sync.dma_start(out=m0i, in_=m0.rearrange("(o l) -> o l", o=1))
    nc.sync.dma_start(out=m1i, in_=m1.rearrange("(o l) -> o l", o=1))
    m0f = consts.tile([1, L], f32r)
    m1f = consts.tile([1, L], f32r)
    nc.vector.tensor_copy(out=m0f, in_=m0i)  # int32 -> fp32 cast
    nc.vector.tensor_copy(out=m1f, in_=m1i)
    nc.vector.tensor_scalar_mul(out=m0f, in0=m0f, scalar1=-MASKC)
    nc.vector.tensor_scalar_mul(out=m1f, in0=m1f, scalar1=MASKC)
    mrows = (m0f, m1f)

    # ---- transpose q0/q1 (data part) to [d-part, l] layout via PE ----
    ident_f = consts.tile([P, P], f32)
    make_identity(nc, ident_f)
    ident = consts.tile([P, P], f32r)
    nc.vector.tensor_copy(out=ident, in_=ident_f)
    q0t = qT.tile([P, DC, L], f32r)
    q1t = qT.tile([P, DC, L], f32r)
    for src, dst in ((q0a, q0t), (q1a, q1t)):
        for t in range(T):
            for dc in range(DC):
                pt = t_psum.tile([P, P], f32r, tag="tp")
                nc.tensor.transpose(pt, src[:, t, dc * P : (dc + 1) * P], ident)
                nc.vector.tensor_copy(out=dst[:, dc, t * P : (t + 1) * P], in_=pt)

    # ---- main phases ----
    # orient 0: rows of E = l0 (feeds out1);  orient 1: rows of E^T = l1 (feeds out0)
    for orient in range(2):
        if orient == 0:
            lT, rT = q0t, q1t
            lm, rm = 0, 1
            raug = q0a
            odram = out1
        else:
            lT, rT = q1t, q0t
            lm, rm = 1, 0
            raug = q1a
            odram = out0
        for h in range(2):
            etiles = []
            for t in range(T):
                ps = s_psum.tile([P, HALF], f32, tag="sp")
                for c in range(HALF // NCHUNK):
                    off = h * HALF + c * NCHUNK
                    sl = ps[:, c * NCHUNK : (c + 1) * NCHUNK]
                    for dc in range(DC):
                        nc.tensor.matmul(
                            sl,
                            lhsT=lT[:, dc, t * P : (t + 1) * P],
                            rhs=rT[:, dc, off : off + NCHUNK],
                            start=(dc == 0),
                            stop=False,
                        )
                    nc.tensor.matmul(
                        sl,
                        lhsT=mrows[lm][:, t * P : (t + 1) * P],
                        rhs=mrows[rm][:, off : off + NCHUNK],
                        start=False,
                        stop=True,
                    )
                et = e_pool.tile([P, HALF], f32r, tag="E")
                nc.scalar.activation(out=et, in_=ps, func=EXP, scale=SCALE2)
                etiles.append(et)
            for mt in range(HALF // P):
                po = o_psum.tile([P, AUGW], f32, tag="op")
                for t in range(T):
                    nc.tensor.matmul(
                        po,
                        lhsT=etiles[t][:, mt * P : (mt + 1) * P],
                        rhs=raug[:, t, :],
                        start=(t == 0),
                        stop=(t == T - 1),
                    )
                rc = small.tile([P, 1], f32, tag="rc")
                nc.vector.reciprocal(rc, po[:, D : D + 1])
                ot = outp.tile([P, D], f32, tag="ot")
                nc.vector.tensor_scalar(
                    out=ot,
                    in0=po[:, 0:D],
                    scalar1=rc,
                    scalar2=SCALE1,
                    op0=MUL,
                    op1=MUL,
                )
                row = h * HALF + mt * P
                nc.sync.dma_start(out=odram[row : row + P, :], in_=ot)


_CACHED_NC = None


def _build():
    global _CACHED_NC
    if _CACHED_NC is not None:
        return _CACHED_NC
    nc = bacc.Bacc("TRN2", target_bir_lowering=False, debug=False)
    io = {
        "q0": nc.dram_tensor("q0", [L, D], f32, kind="ExternalInput").ap(),
        "q1": nc.dram_tensor("q1", [L, D], f32, kind="ExternalInput").ap(),
        "mask0": nc.dram_tensor("mask0", [L], i32, kind="ExternalInput").ap(),
        "mask1": nc.dram_tensor("mask1", [L], i32, kind="ExternalInput").ap(),
        "out0": nc.dram_tensor("out0", [L, D], f32, kind="ExternalOutput").ap(),
        "out1": nc.dram_tensor("out1", [L, D], f32, kind="ExternalOutput").ap(),
    }
    with tile.TileContext(nc) as tc:
        with ExitStack() as ctx:
            _emit(tc, ctx, io)
    nc.compile()
    _CACHED_NC = nc
    return nc


def run_on_cores(q0, q1, mask0, mask1, trace=False):
    """Run the SPMD kernel; returns (out0, out1, BassKernelResults)."""
    nc = _build()
    in_maps = [
        {
            "q0": np.ascontiguousarray(q0[b], dtype=np.float32),
            "q1": np.ascontiguousarray(q1[b], dtype=np.float32),
            "mask0": np.ascontiguousarray(mask0[b], dtype=np.int32),
            "mask1": np.ascontiguousarray(mask1[b], dtype=np.int32),
        }
        for b in range(B)
    ]
    br = run_bass_kernel_spmd(nc, in_maps, list(range(B)), trace=trace)
    out0 = np.stack([br.results[b]["out0"] for b in range(B)])
    out1 = np.stack([br.results[b]["out1"] for b in range(B)])
    return out0, out1, br


def kernel(q0, q1, len0=None, len1=None, mask0=None, mask1=None, **_):
    q0 = np.asarray(q0, dtype=np.float32)
    q1 = np.asarray(q1, dtype=np.float32)
    mask0 = np.asarray(mask0, dtype=np.int32)
    mask1 = np.asarray(mask1, dtype=np.int32)
    out0, out1, _br = run_on_cores(q0, q1, mask0, mask1, trace=False)
    return out0, out1

